# revision 1
# baseline (speedup 1.0000x reference)
"""Trainium2 Bass kernel for DGCNN (gnn_message_passing).

Strategy: data-parallel over graphs — 32 graphs per NeuronCore x 8 cores,
no collectives. Per core:
  1. embedding via dma_gather from z_table
  2. per-graph dense edge-count matrix Ct[s, d] built once on-device:
     DVE one-hot rows (is_equal vs iota) + dma_scatter_add (CCE accumulate)
  3. 3 GraphConv layers as dense 128x128 PE matmuls; degrees from Ct
     (row-reduce for deg_out, ones-column matmul for deg_in); rsqrt via
     ACT sqrt + DVE reciprocal + one Newton step; tanh on ACT
  4. SortPooling: per-layer row-max, per-graph top-30 via max8/max_index/
     match_replace, gather the 960 selected rows, bitonic-sort 384 feats
  5. conv1/maxpool/conv2/lin1/lin2 tail as small PE matmuls

kernel(**inputs) takes FULL unsharded inputs (as in reference.setup_inputs),
shards on host (index marshalling only), runs the same Bass program SPMD on
cores 0-7 with per-core input shards, and concatenates per-core outputs.
"""

import os

import numpy as np

import concourse.bass as bass
import concourse.bacc as bacc
import concourse.mybir as mybir
import concourse.tile as tile
from concourse.masks import make_identity

F32 = mybir.dt.float32
BF16 = mybir.dt.bfloat16
I16 = mybir.dt.int16
U16 = mybir.dt.uint16
ALU = mybir.AluOpType
ACTF = mybir.ActivationFunctionType
AX = mybir.AxisListType

B, NPER, DEG, H, L, K = 256, 256, 16, 128, 3, 30
NCORES = 8
GPC = B // NCORES            # 32 graphs per core
NPC = GPC * NPER             # 8192 nodes per core
EPC = NPC * DEG              # 131072 edges per core
NBLK = NPC // 128            # 64 node blocks of 128
CT_ROWS = GPC * 512          # 16384 scatter rows (g, d_hi, s)
DUMMY_KEY = CT_ROWS          # sacrificial row for round padding
SC_CHUNK = 4096             # edges per scatter chunk (SWDGE ring <= 512 descs)
N_CHUNK = EPC // SC_CHUNK    # 16 chunks
NEG = -2.0                   # below any tanh output
PAD = 1.0e30                 # sorts to the end (ascending)
NROWS = GPC * K              # 960 pooled rows
RBLK = 8                     # ceil(960/128) row blocks


def build_nc():
    phase = int(os.environ.get("KERNEL_PHASE", "9"))
    nc = bacc.Bacc(None)

    # ---- inputs (host-marshalled shards; see shard_inputs) ----
    z_idx = nc.dram_tensor("z_idx", [128, NPC // 16], I16, kind="ExternalInput")
    s_arr = nc.dram_tensor("s_arr", [128, EPC // 128], BF16, kind="ExternalInput")
    d_arr = nc.dram_tensor("d_arr", [128, EPC // 128], BF16, kind="ExternalInput")
    z_table = nc.dram_tensor("z_table", [1000, H], F32, kind="ExternalInput")
    biases = nc.dram_tensor("biases", [L, H], F32, kind="ExternalInput")
    w1 = nc.dram_tensor("w1", [16, 384], F32, kind="ExternalInput")
    b1 = nc.dram_tensor("b1", [16, 1], F32, kind="ExternalInput")
    w2m = nc.dram_tensor("w2m", [32, 80], F32, kind="ExternalInput")
    b2 = nc.dram_tensor("b2", [32, 1], F32, kind="ExternalInput")
    lw1m = nc.dram_tensor("lw1m", [128, 352], F32, kind="ExternalInput")
    lb1 = nc.dram_tensor("lb1", [128, 1], F32, kind="ExternalInput")
    lw2 = nc.dram_tensor("lw2", [128, 1], F32, kind="ExternalInput")
    lb2 = nc.dram_tensor("lb2", [1, 1], F32, kind="ExternalInput")
    out_d = nc.dram_tensor("out", [GPC, 1], F32, kind="ExternalOutput")

    # ---- DRAM scratch ----
    x_d = [nc.dram_tensor(f"x{l}_scratch", [NPC, H], F32, kind="Internal")
           for l in range(L)]
    ids_d = nc.dram_tensor("ids_scratch", [NROWS], I16, kind="Internal")
    idsw_d = nc.dram_tensor("idsw_scratch", [128, NROWS // 16], I16,
                            kind="Internal")

    with tile.TileContext(nc) as tc:
        with (
            tc.tile_pool(name="big", bufs=1) as big,
            tc.tile_pool(name="work", bufs=2) as work,
            tc.tile_pool(name="small", bufs=1) as small,
            tc.tile_pool(name="psum", bufs=2, space="PSUM") as psum,
            tc.tile_pool(name="psumcb", bufs=4, space="PSUM") as psumcb,
            tc.tile_pool(name="psum1", bufs=1, space="PSUM") as psum1,
        ):
            # ---------- constants ----------
            ident = small.tile([128, 128], F32, tag="ident")
            make_identity(nc, ident[:])
            iota_i16 = small.tile([128, 256], I16, tag="iota16")
            nc.gpsimd.iota(iota_i16[:], pattern=[[1, 256]], base=0,
                           channel_multiplier=0)
            iota_bf = small.tile([128, 256], BF16, tag="iotabf")
            nc.vector.tensor_copy(iota_bf[:], iota_i16[:])

            # ---------- embedding gather: h = z_table[z] ----------
            h_buf = big.tile([128, NBLK, H], F32, tag="h")
            zi = small.tile([128, NPC // 16], I16, tag="zi")
            nc.sync.dma_start(out=zi[:], in_=z_idx[:])
            nc.gpsimd.dma_gather(
                out_ap=h_buf[:], in_ap=z_table[:], idxs_ap=zi[:],
                num_idxs=NPC, num_idxs_reg=NPC, elem_size=H,
                single_packet=False)

            # ---------- C build: PE one-hot matmuls ----------
            # Per half-graph chunk (2048 edges, 16 blocks of 128): build
            # one-hot(s) and one-hot(d) [128, 16, 256] bf16 on DVE, then
            # accumulate C[s, d] = sum_e oh_s[e,s] * oh_d[e,d] in PSUM.
            # ct_sb groups: idx = g*2 + s_half; content [s%128, d (256)].
            ct_sb = big.tile([128, 2 * GPC, 256], BF16, tag="ct")
            s_all = small.tile([128, EPC // 128], BF16, tag="s_all")
            d_all = small.tile([128, EPC // 128], BF16, tag="d_all")
            nc.sync.dma_start(out=s_all[:], in_=s_arr[:])
            nc.sync.dma_start(out=d_all[:], in_=d_arr[:])
            EBLK = 8             # edge blocks per chunk (1024 edges)
            for g in range(GPC):
                pcs = [psumcb.tile([128, 256], F32, tag="cb",
                                   name=f"cb{g}_{s}") for s in range(2)]
                for q in range(4):
                    ci = g * 4 + q
                    oh_s = work.tile([128, EBLK, 256], BF16, tag="oh_s",
                                     name=f"oh_s{ci}")
                    oh_d = work.tile([128, EBLK, 256], BF16, tag="oh_d",
                                     name=f"oh_d{ci}")
                    nc.vector.tensor_tensor(
                        out=oh_s[:],
                        in0=s_all[:, ci * EBLK:(ci + 1) * EBLK].unsqueeze(2)
                        .broadcast_to([128, EBLK, 256]),
                        in1=iota_bf[:].unsqueeze(1).broadcast_to(
                            [128, EBLK, 256]),
                        op=ALU.is_equal)
                    nc.vector.tensor_tensor(
                        out=oh_d[:],
                        in0=d_all[:, ci * EBLK:(ci + 1) * EBLK].unsqueeze(2)
                        .broadcast_to([128, EBLK, 256]),
                        in1=iota_bf[:].unsqueeze(1).broadcast_to(
                            [128, EBLK, 256]),
                        op=ALU.is_equal)
                    for b in range(EBLK):
                        for ss in range(2):
                            nc.tensor.matmul(
                                out=pcs[ss][:],
                                lhsT=oh_s[:, b, ss * 128:(ss + 1) * 128],
                                rhs=oh_d[:, b, :],
                                start=(q == 0 and b == 0),
                                stop=(q == 3 and b == EBLK - 1))
                for ss in range(2):
                    nc.scalar.copy(ct_sb[:, g * 2 + ss, :], pcs[ss][:])

            if phase >= 2:
                # ---------- degrees ----------
                dga = small.tile([128, NBLK], F32, tag="dga")
                nsrc = small.tile([128, NBLK], F32, tag="nsrc")
                ndst = small.tile([128, NBLK], F32, tag="ndst")
                # deg_out[(g,s-half) blk] = sum over the 256 d cols
                nc.vector.tensor_reduce(
                    out=dga[:].rearrange("p (g s) -> p g s", g=GPC),
                    in_=ct_sb[:].rearrange("p (g s) m -> p g s m", g=GPC),
                    axis=AX.X, op=ALU.add)
                _rsqrt(nc, small, nsrc, dga, "a")  # nsrc = rsqrt(max(deg_out,1))

                # ---------- layers ----------
                hpre = big.tile([128, NBLK, H + 1], F32, tag="hpre")
                rmax = [small.tile([128, NBLK], F32, tag=f"rmax{l}", name=f"rmax{l}")
                        for l in range(L)]
                bias_rep = small.tile([128, H], F32, tag="brep")
                deg_in = small.tile([128, NBLK], F32, tag="degin")

                for l in range(L):
                    ncol = H + 1 if l == 0 else H
                    if l == 0:
                        nc.vector.memset(hpre[:, :, H:H + 1], 1.0)
                    # h_pre = h * nsrc (broadcast nsrc along H)
                    nc.vector.tensor_mul(
                        out=hpre[:, :, 0:H], in0=h_buf[:],
                        in1=nsrc[:].unsqueeze(2).broadcast_to(
                            [128, NBLK, H]))
                    # agg[d, :] = sum_s Ct[s, d] * h_pre[s, :]
                    for g in range(GPC):
                        ctf = work.tile([128, 2, 256], F32, tag="ctf",
                                        name=f"ctf{l}_{g}")
                        nc.scalar.copy(ctf[:],
                                       ct_sb[:, g * 2:(g + 1) * 2, :])
                        pt = [psum.tile([128, ncol], F32, tag="mm",
                                        name=f"mm{g}_{_i}")
                              for _i in range(2)]
                        for so in range(2):  # d half
                            for si in range(2):  # s half
                                nc.tensor.matmul(
                                    out=pt[so][:],
                                    lhsT=ctf[:, si, so * 128:(so + 1) * 128],
                                    rhs=hpre[:, 2 * g + si, 0:ncol],
                                    start=(si == 0), stop=(si == 1))
                        for so in range(2):
                            nc.scalar.copy(hpre[:, 2 * g + so, 0:ncol],
                                           pt[so][:])
                    if l == 0:
                        nc.vector.tensor_copy(
                            deg_in[:], hpre[:, :, H:H + 1].squeeze(2))
                        _rsqrt(nc, small, ndst, deg_in, "b")
                    # bias replicate: bias_rep[p, j] = biases[l, j]
                    nc.sync.dma_start(
                        out=bias_rep[:],
                        in_=biases[l:l + 1, :].broadcast_to([128, H]))
                    # h = tanh(agg * ndst + bias)
                    nc.vector.tensor_mul(
                        out=hpre[:, :, 0:H], in0=hpre[:, :, 0:H],
                        in1=ndst[:].unsqueeze(2).broadcast_to(
                            [128, NBLK, H]))
                    nc.vector.tensor_add(
                        out=hpre[:, :, 0:H], in0=hpre[:, :, 0:H],
                        in1=bias_rep[:].unsqueeze(1).broadcast_to([128, NBLK, H]))
                    nc.scalar.activation(h_buf[:], hpre[:, :, 0:H], ACTF.Tanh)
                    # row-max over H, spill h to DRAM
                    nc.vector.tensor_reduce(out=rmax[l][:], in_=h_buf[:],
                                            axis=AX.X, op=ALU.max)
                    nc.sync.dma_start(
                        out=x_d[l][:].rearrange("(b p) m -> p b m", p=128),
                        in_=h_buf[:])

            if phase >= 3:
                # ---------- top-30 per graph ----------
                nc.vector.tensor_tensor(out=rmax[0][:], in0=rmax[0][:],
                                        in1=rmax[1][:], op=ALU.max)
                nc.vector.tensor_tensor(out=rmax[0][:], in0=rmax[0][:],
                                        in1=rmax[2][:], op=ALU.max)
                # per-sigma transpose: rm[:, s::2] (128, 32) -> (32, 128)
                gm = small.tile([GPC, NPER], F32, tag="gm")
                for s in range(2):
                    ptr = psum1.tile([GPC, 128], F32, tag="tp2")
                    nc.tensor.transpose(
                        out=ptr[:],
                        in_=rmax[0][:].rearrange("p (g s) -> p s g", s=2)[:, s],
                        identity=ident[:])
                    nc.vector.tensor_copy(gm[:, s * 128:(s + 1) * 128], ptr[:])
                # iterative top-32 (use first 30)
                ids = small.tile([GPC, 32], U16, tag="ids")
                vals8 = small.tile([GPC, 8], F32, tag="vals8")
                for r in range(4):
                    nc.vector.max(out=vals8[:], in_=gm[:])
                    nc.vector.max_index(out=ids[:, r * 8:(r + 1) * 8],
                                        in_max=vals8[:], in_values=gm[:])
                    nc.vector.match_replace(out=gm[:], in_to_replace=vals8[:],
                                            in_values=gm[:], imm_value=NEG)
                # global node id = g*256 + idx
                gid = small.tile([GPC, 32], I16, tag="gid")
                goff = small.tile([GPC, 1], I16, tag="goff")
                nc.gpsimd.iota(goff[:], pattern=[[1, 1]], base=0,
                               channel_multiplier=NPER)
                nc.vector.tensor_tensor(out=gid[:], in0=ids[:],
                                        in1=goff[:].broadcast_to([GPC, 32]),
                                        op=ALU.add)
                nc.sync.dma_start(
                    out=ids_d[:].rearrange("(g k) -> g k", g=GPC),
                    in_=gid[:, 0:K])
                # reload in [16, NROWS/16] gather wrap, replicate to 128 parts
                gw16 = small.tile([16, NROWS // 16], I16, tag="gw16")
                nc.sync.dma_start(
                    out=gw16[:], in_=ids_d[:].rearrange("(t p) -> p t", p=16))
                for r in range(8):
                    nc.sync.dma_start(out=idsw_d[r * 16:(r + 1) * 16, :],
                                      in_=gw16[:])
                gidx = small.tile([128, NROWS // 16], I16, tag="gidx")
                nc.sync.dma_start(out=gidx[:], in_=idsw_d[:])

                # ---------- gather pooled rows + sort ----------
                xs = big.tile([128, RBLK, 512], F32, tag="xs")
                nc.vector.memset(xs[:], PAD)
                for l in range(L):
                    gx = work.tile([128, RBLK, H], F32, tag="sc")
                    nc.vector.memset(gx[:], 0.0)
                    nc.gpsimd.dma_gather(
                        out_ap=gx[:], in_ap=x_d[l][:], idxs_ap=gidx[:],
                        num_idxs=NROWS, num_idxs_reg=NROWS, elem_size=H)
                    nc.vector.tensor_copy(xs[:, :, l * H:(l + 1) * H], gx[:])
                xs2 = work.tile([128, RBLK, 512], F32, tag="sc")
                xs_fin = _bitonic_sort(nc, xs, xs2)

            if phase >= 4:
                # ---------- conv tail ----------
                # transpose pooled -> pooledT [d (3x128), rows (8x128)]
                pooled_t = [small.tile([128, RBLK * 128], F32, tag=f"pt{j}", name=f"pt{j}")
                            for j in range(3)]
                for j in range(3):
                    for bb in range(RBLK):
                        ptr = psum1.tile([128, 128], F32, tag="tp2")
                        nc.tensor.transpose(
                            out=ptr[:], in_=xs_fin[:, bb, j * 128:(j + 1) * 128],
                            identity=ident[:])
                        nc.scalar.copy(
                            pooled_t[j][:, bb * 128:(bb + 1) * 128], ptr[:])
                # weights in
                w1_sb = small.tile([16, 384], F32, tag="w1")
                nc.sync.dma_start(out=w1_sb[:], in_=w1[:])
                w1t = small.tile([128, 3, 16], F32, tag="w1t")
                for j in range(3):
                    ptr = psum1.tile([128, 16], F32, tag="tp2")
                    nc.tensor.transpose(out=ptr[:],
                                        in_=w1_sb[:, j * 128:(j + 1) * 128],
                                        identity=ident[:16, :16])
                    nc.scalar.copy(w1t[:, j, :], ptr[:])
                b1_sb = small.tile([16, 1], F32, tag="b1")
                nc.sync.dma_start(out=b1_sb[:], in_=b1[:])
                # conv1: out1[o, r] = relu(sum_d w1[o,d] pooled[r,d] + b1[o])
                out1 = small.tile([16, RBLK * 128], F32, tag="out1")
                for ch in range(2):
                    pc = psum1.tile([16, 512], F32, tag="cc")
                    for j in range(3):
                        nc.tensor.matmul(
                            out=pc[:], lhsT=w1t[:, j, :],
                            rhs=pooled_t[j][:, ch * 512:(ch + 1) * 512],
                            start=(j == 0), stop=(j == 2))
                    nc.scalar.activation(out1[:, ch * 512:(ch + 1) * 512], pc[:],
                                         ACTF.Relu, bias=b1_sb[:, 0:1])
                # maxpool pairs over k: out1 cols are (g,k) = g*30+k, k<30
                pmax = small.tile([16, GPC * 15], F32, tag="pmax")
                o1v = out1[:, 0:GPC * 30].rearrange("p (g k) -> p g k", g=GPC)
                o1v = o1v.rearrange("p g (i two) -> p g i two", two=2)
                nc.vector.tensor_tensor(
                    out=pmax[:].rearrange("p (g i) -> p g i", g=GPC),
                    in0=o1v[:, :, :, 0], in1=o1v[:, :, :, 1], op=ALU.max)
                # conv2: accumulate over dt taps; lhsT per tap = w2t_dt [16, 32]
                w2_sb = small.tile([32, 80], F32, tag="w2")
                nc.sync.dma_start(out=w2_sb[:], in_=w2m[:])
                w2t = small.tile([16, 5, 32], F32, tag="w2t")
                for dt in range(5):
                    ptr = psum1.tile([16, 32], F32, tag="tp2")
                    nc.tensor.transpose(out=ptr[:],
                                        in_=w2_sb[:, dt * 16:(dt + 1) * 16],
                                        identity=ident[:32, :32])
                    nc.scalar.copy(w2t[:, dt, :], ptr[:])
                b2_sb = small.tile([32, 1], F32, tag="b2")
                nc.sync.dma_start(out=b2_sb[:], in_=b2[:])
                out2 = small.tile([32, GPC * 11], F32, tag="out2")
                pv = pmax[:].rearrange("p (g i) -> p g i", g=GPC)
                pc2 = psum1.tile([32, GPC * 11], F32, tag="cc")
                for dt in range(5):
                    nc.tensor.matmul(
                        out=pc2[:].rearrange("p (g t) -> p g t", g=GPC),
                        lhsT=w2t[:, dt, :], rhs=pv[:, :, dt:dt + 11],
                        start=(dt == 0), stop=(dt == 4))
                nc.scalar.activation(out2[:], pc2[:], ACTF.Relu,
                                     bias=b2_sb[:, 0:1])
                # lin1 rhs: rhs352[(t,oc), g] = out2[oc, g*11+t], 3 K-tiles
                rhs352 = [small.tile([128, GPC], F32, tag=f"rhs352_{j}",
                                     name=f"rhs352_{j}") for j in range(3)]
                o2v = out2[:].rearrange("p (g t) -> p g t", g=GPC)
                for t in range(11):
                    j, r = t // 4, (t % 4) * 32
                    nc.vector.tensor_copy(rhs352[j][r:r + 32], o2v[:, :, t])
                lw1_sb = small.tile([128, 352], F32, tag="lw1")
                nc.sync.dma_start(out=lw1_sb[:], in_=lw1m[:])
                lw1t = [small.tile([128, 128], F32, tag=f"lw1t{j}", name=f"lw1t{j}")
                        for j in range(3)]
                for j in range(3):
                    w = 128 if j < 2 else 96
                    ptr = psum1.tile([128, 128], F32, tag="tp2")
                    nc.tensor.transpose(out=ptr[:w, :],
                                        in_=lw1_sb[:, j * 128:j * 128 + w],
                                        identity=ident[:])
                    nc.scalar.copy(lw1t[j][:w, :], ptr[:w, :])
                lb1_sb = small.tile([128, 1], F32, tag="lb1")
                nc.sync.dma_start(out=lb1_sb[:], in_=lb1[:])
                h1t = small.tile([128, GPC], F32, tag="h1t")
                pc3 = psum1.tile([128, GPC], F32, tag="cc")
                for j in range(3):
                    w = 128 if j < 2 else 96
                    nc.tensor.matmul(out=pc3[:], lhsT=lw1t[j][:w, :],
                                     rhs=rhs352[j][:w, :],
                                     start=(j == 0), stop=(j == 2))
                nc.scalar.activation(h1t[:], pc3[:], ACTF.Relu,
                                     bias=lb1_sb[:, 0:1])
                # lin2
                lw2_sb = small.tile([128, 1], F32, tag="lw2")
                nc.sync.dma_start(out=lw2_sb[:], in_=lw2[:])
                lb2_sb = small.tile([GPC, 1], F32, tag="lb2")
                nc.sync.dma_start(out=lb2_sb[:],
                                  in_=lb2[:].broadcast_to([GPC, 1]))
                pc4 = psum1.tile([GPC, 1], F32, tag="cc")
                nc.tensor.matmul(out=pc4[:], lhsT=h1t[:], rhs=lw2_sb[:],
                                 start=True, stop=True)
                res = small.tile([GPC, 1], F32, tag="res")
                nc.vector.tensor_add(out=res[:], in0=pc4[:], in1=lb2_sb[:])
                nc.sync.dma_start(out=out_d[:], in_=res[:])

            if phase < 4:
                dbg = small.tile([GPC, 1], F32, tag="dbg")
                if phase >= 3:
                    nc.vector.tensor_copy(dbg[:], xs_fin[0:GPC, 0, 0:1])
                else:
                    nc.vector.tensor_copy(dbg[:], h_buf[0:GPC, 0, 0:1])
                nc.sync.dma_start(out=out_d[:], in_=dbg[:])
    nc.compile()
    return nc


def _rsqrt(nc, pool, out_t, deg_t, tg):
    """out = rsqrt(max(deg, 1)) : ACT sqrt + DVE reciprocal + 1 Newton step."""
    shape = [deg_t.shape[0], deg_t.shape[1]]
    d1 = pool.tile(shape, F32, tag=f"rs_d{tg}")
    s = pool.tile(shape, F32, tag=f"rs_s{tg}")
    t = pool.tile(shape, F32, tag=f"rs_t{tg}")
    nc.vector.tensor_scalar_max(d1[:], deg_t[:], 1.0)
    nc.scalar.sqrt(s[:], d1[:])
    nc.vector.reciprocal(out_t[:], s[:])
    # r = r * (1.5 - 0.5 * d * r * r)
    nc.vector.tensor_mul(out=t[:], in0=out_t[:], in1=out_t[:])
    nc.vector.tensor_mul(out=t[:], in0=t[:], in1=d1[:])
    nc.vector.tensor_scalar(out=t[:], in0=t[:], scalar1=-0.5, scalar2=1.5,
                            op0=ALU.mult, op1=ALU.add)
    nc.vector.tensor_mul(out=out_t[:], in0=out_t[:], in1=t[:])


def _bitonic_sort(nc, xs, xs2):
    """Ascending bitonic sort along last axis (512) of [128, RBLK, 512].

    Ping-pong between xs and xs2; returns the buffer holding the result.
    """
    bufs = [xs, xs2]
    cur = 0

    def stage(src, dst, lo_idx, hi_idx):
        nc.vector.tensor_tensor(out=lo_idx(dst), in0=lo_idx(src),
                                in1=hi_idx(src), op=ALU.min)
        nc.vector.tensor_tensor(out=hi_idx(dst), in0=lo_idx(src),
                                in1=hi_idx(src), op=ALU.max)

    for k in range(1, 10):
        bs = 1 << k
        half = bs // 2

        def flip_lo(t, bs=bs, half=half):
            return t[:].rearrange("p r (b i) -> p (r b) i", i=bs)[:, :, 0:half]

        def flip_hi(t, bs=bs, half=half):
            v = t[:].rearrange("p r (b i) -> p (r b) i", i=bs)
            return v[:, :, bs - 1:half - 1:-1]

        stage(bufs[cur], bufs[1 - cur], flip_lo, flip_hi)
        cur = 1 - cur
        for d in [1 << j for j in range(k - 2, -1, -1)]:
            def cl_lo(t, d=d):
                return t[:].rearrange("p r (b two i) -> p (r b) two i",
                                      two=2, i=d)[:, :, 0]

            def cl_hi(t, d=d):
                return t[:].rearrange("p r (b two i) -> p (r b) two i",
                                      two=2, i=d)[:, :, 1]

            stage(bufs[cur], bufs[1 - cur], cl_lo, cl_hi)
            cur = 1 - cur
    return bufs[cur]


# ======================= host side =======================

_NC_CACHE = []


def _get_nc():
    if not _NC_CACHE:
        _NC_CACHE.append(build_nc())
    return _NC_CACHE[0]


def _wrap(a, p):
    """Return [p, len(a)//p] with element i at [i % p, i // p]."""
    return np.ascontiguousarray(a.reshape(-1, p).T)


def shard_inputs(inputs):
    z = np.asarray(inputs["z"]).astype(np.int64)
    src = np.asarray(inputs["src"]).astype(np.int64)
    dst = np.asarray(inputs["dst"]).astype(np.int64)
    z_table = np.ascontiguousarray(np.asarray(inputs["z_table"], np.float32))
    biases = np.ascontiguousarray(np.asarray(inputs["biases"], np.float32))
    w1 = np.ascontiguousarray(
        np.asarray(inputs["conv1_w"], np.float32).reshape(16, 384))
    b1 = np.ascontiguousarray(
        np.asarray(inputs["conv1_b"], np.float32).reshape(16, 1))
    w2m = np.ascontiguousarray(
        np.asarray(inputs["conv2_w"], np.float32).transpose(0, 2, 1)
        .reshape(32, 80))
    b2 = np.ascontiguousarray(
        np.asarray(inputs["conv2_b"], np.float32).reshape(32, 1))
    lw1m = np.ascontiguousarray(
        np.asarray(inputs["lin1_w"], np.float32).reshape(128, 32, 11)
        .transpose(0, 2, 1).reshape(128, 352))
    lb1 = np.ascontiguousarray(
        np.asarray(inputs["lin1_b"], np.float32).reshape(128, 1))
    lw2 = np.ascontiguousarray(
        np.asarray(inputs["lin2_w"], np.float32).reshape(128, 1))
    lb2 = np.ascontiguousarray(
        np.asarray(inputs["lin2_b"], np.float32).reshape(1, 1))

    bf16 = mybir.dt.np(BF16)
    in_maps = []
    for c in range(NCORES):
        se = src[c * EPC:(c + 1) * EPC] - c * NPC
        de = dst[c * EPC:(c + 1) * EPC] - c * NPC
        s_g = se % NPER
        d_g = de % NPER
        zl = z[c * NPC:(c + 1) * NPC]
        in_maps.append({
            "z_idx": np.tile(_wrap(zl.astype(np.int16), 16), (8, 1)),
            "s_arr": _wrap(s_g.astype(np.int16), 128).astype(bf16),
            "d_arr": _wrap(d_g.astype(np.int16), 128).astype(bf16),
            "z_table": z_table, "biases": biases,
            "w1": w1, "b1": b1, "w2m": w2m, "b2": b2,
            "lw1m": lw1m, "lb1": lb1, "lw2": lw2, "lb2": lb2,
        })
    return in_maps


def kernel(**inputs):
    from concourse.bass_utils import run_bass_kernel_spmd
    in_maps = shard_inputs(inputs)
    nc = _get_nc()
    res = run_bass_kernel_spmd(nc, in_maps, core_ids=list(range(NCORES)))
    outs = [np.asarray(res.results[c]["out"], np.float32)
            for c in range(NCORES)]
    return np.concatenate(outs, axis=0)

